# revision 11
# baseline (speedup 1.0000x reference)
"""Trainium2 Bass kernel for a full attention layer (QKV proj + interleaved
RoPE + non-causal SDPA + output proj), tensor-parallel over heads on 8
NeuronCores.

Hardcoded problem shape: B=2, S=2048, HID=2048, H=16 heads, DH=128, fp32.

Sharding (per core c of 8): heads 2c, 2c+1.
 - w_qkv rows for those heads (q/k rows de-interleaved per head so RoPE's
   (2i, 2i+1) pairing becomes a 64-partition block swap), transposed to
   [HID, 256] so the contraction dim (HID) rides the SBUF partition axis.
 - w_o columns for those heads, transposed to [256, HID].
 - hidden_states transposed to [HID, B*S] (replicated to every core).
 - cos/sin prepped as de-interleaved, transposed [128, S] tiles; sin carries
   the rotate-half sign in its first 64 rows.
Each core computes a full-shape partial output [B*S, HID] (its heads'
contribution through w_o) in bf16; the host unshards by summing the 8
partials in fp32.

All matmuls run as float32r (full PE rate for moving dim >= 256; fp32 data).
Attention is computed in the S^T orientation: scores come out as
P^T[k, q] tiles so the AV matmul can contract k on the partition axis with
no transposes anywhere.  The softmax denominator is an all-ones [128,128]
stationary matmul accumulated alongside AV; out tiles are scaled by its
approx reciprocal after AV (divide-after-AV).  exp() is fused into the
PSUM->SBUF drain on the scalar engine over two PSUM banks at a time, with
the 1/sqrt(DH) scale folded in.  No max-subtraction: scores are ~N(0,1) so
exp is safe in fp32.  RoPE runs on the vector engine directly out of PSUM
using cross-partition-offset operands (no swap DMAs, no scalar copies).
DMA dispatch is split across engine queues: weights on the scalar queue,
cos/sin on gpsimd, activations + output stores on sync.
"""

import os

import numpy as np

B, S, HID = 2, 2048, 2048
H, DH = 16, 128
NC = 8
HPC = H // NC          # heads per core = 2
OC = HPC * DH          # per-core o width per section = 256
T = B * S              # 4096 tokens
KT = HID // 128        # 16 contraction tiles
TC = 256               # token chunk for QKV projection
QC = 512               # query chunk for attention
SCALE = 1.0 / float(np.sqrt(DH))

_exec_time_ns = None   # stashed by kernel() for the test harness


R_EXP2 = bool(int(os.environ.get("R_EXP2", "0")))    # Exp over 2-bank PSUM span
R_ROPE = bool(int(os.environ.get("R_ROPE", "0")))    # DVE cross-partition RoPE from PSUM
R_RECIP = bool(int(os.environ.get("R_RECIP", "1")))  # reciprocal_approx_fast
R_DMAQ = bool(int(os.environ.get("R_DMAQ", "1")))    # split DMA dispatch queues


def _build():
    import concourse.bacc as bacc
    import concourse.mybir as mybir
    import concourse.tile as tile

    f32 = mybir.dt.float32
    fr = mybir.dt.float32r
    bf16 = mybir.dt.bfloat16
    Exp = mybir.ActivationFunctionType.Exp

    nc = bacc.Bacc("TRN2", target_bir_lowering=False)

    hT = nc.dram_tensor("hT", [HID, T], fr, kind="ExternalInput")
    wqT = nc.dram_tensor("wqT", [HID, OC], fr, kind="ExternalInput")
    wkT = nc.dram_tensor("wkT", [HID, OC], fr, kind="ExternalInput")
    wvT = nc.dram_tensor("wvT", [HID, OC], fr, kind="ExternalInput")
    woT = nc.dram_tensor("woT", [OC, HID], fr, kind="ExternalInput")
    cc = nc.dram_tensor("cc", [DH, S], f32, kind="ExternalInput")
    ss = nc.dram_tensor("ss", [DH, S], f32, kind="ExternalInput")
    out_p = nc.dram_tensor("out_p", [T, HID], bf16, kind="ExternalOutput")

    hT_r = hT.rearrange("(k p) t -> p k t", p=128)      # [128, 16, T]
    wqT_r = wqT.rearrange("(k p) o -> p k o", p=128)    # [128, 16, 256]
    wkT_r = wkT.rearrange("(k p) o -> p k o", p=128)
    wvT_r = wvT.rearrange("(k p) o -> p k o", p=128)
    woT_r = woT.rearrange("(h p) n -> p h n", p=128)    # [128, 2, 2048]

    with tile.TileContext(nc) as tc:
        with (
            tc.tile_pool(name="const", bufs=1) as constp,
            tc.tile_pool(name="hbuf", bufs=2) as hpool,
            tc.tile_pool(name="qkv", bufs=1) as qkvp,
            tc.tile_pool(name="rope", bufs=2) as ropep,
            tc.tile_pool(name="pbuf", bufs=3) as pp,
            tc.tile_pool(name="small", bufs=2) as smallp,
            tc.tile_pool(name="fout", bufs=2) as foutp,
        ):
            # ---- resident weights/constants: one DMA each, on side queues ----
            weng = nc.scalar if R_DMAQ else nc.sync
            ceng = nc.gpsimd if R_DMAQ else nc.sync
            wq_sb = constp.tile([128, KT, OC], fr)
            wk_sb = constp.tile([128, KT, OC], fr)
            wv_sb = constp.tile([128, KT, OC], fr)
            weng.dma_start(out=wq_sb, in_=wqT_r)
            weng.dma_start(out=wk_sb, in_=wkT_r)
            weng.dma_start(out=wv_sb, in_=wvT_r)
            wo_sb = constp.tile([128, HPC, HID], fr)
            weng.dma_start(out=wo_sb, in_=woT_r)
            cc_sb = constp.tile([128, S], f32)
            ss_sb = constp.tile([128, S], f32)
            ceng.dma_start(out=cc_sb, in_=cc[:, :])
            ceng.dma_start(out=ss_sb, in_=ss[:, :])
            ones_f32 = constp.tile([128, 128], f32)
            nc.vector.memset(ones_f32, 1.0)
            ones_sb = constp.tile([128, 128], fr)
            nc.vector.tensor_copy(ones_sb, ones_f32)

            w_of = [(wq_sb, 0), (wq_sb, 1), (wk_sb, 0), (wk_sb, 1)]

            for b in range(B):
                t0 = b * S

                # ---- phase 1: QKV projection (+ fused RoPE for q,k) ----
                # qk_sb rows: [q_h0, q_h1, k_h0, k_h1], each [128 d, S]
                qk_sb = qkvp.tile([128, 4, S], fr, tag="qk")
                v_sb = qkvp.tile([128, KT, OC], fr, tag="v")
                with tc.tile_pool(name="ps1", bufs=4, space="PSUM") as ps1:
                    for tci in range(S // TC):
                        soff = tci * TC
                        hch = hpool.tile([128, KT, TC], fr, tag="hch")
                        nc.sync.dma_start(
                            out=hch, in_=hT_r[:, :, t0 + soff : t0 + soff + TC]
                        )
                        for ot in range(4):
                            wsb, hl = w_of[ot]
                            ps = ps1.tile([128, TC], f32, tag="ps_qk")
                            for kk in range(KT):
                                nc.tensor.matmul(
                                    ps,
                                    wsb[:, kk, hl * DH : (hl + 1) * DH],
                                    hch[:, kk, :],
                                    start=(kk == 0),
                                    stop=(kk == KT - 1),
                                )
                            # RoPE: dst = ps*cc + blockswap(ps)*ss_signed
                            dst = qk_sb[:, ot, soff : soff + TC]
                            if R_ROPE:
                                # DVE reads PSUM directly, cross-partition
                                t2 = ropep.tile([128, TC], f32, tag="t2")
                                nc.vector.tensor_mul(
                                    t2[0:64, :],
                                    ps[64:128, :],
                                    ss_sb[0:64, soff : soff + TC],
                                )
                                nc.vector.tensor_mul(
                                    t2[64:128, :],
                                    ps[0:64, :],
                                    ss_sb[64:128, soff : soff + TC],
                                )
                                t1 = ropep.tile([128, TC], f32, tag="t1")
                                nc.vector.tensor_mul(
                                    t1, ps, cc_sb[:, soff : soff + TC]
                                )
                                nc.vector.tensor_add(dst, t1, t2)
                            else:
                                raw = ropep.tile([128, TC], f32, tag="raw")
                                nc.scalar.copy(raw, ps)
                                swp = ropep.tile([128, TC], f32, tag="swp")
                                nc.sync.dma_start(
                                    out=swp[0:64, :], in_=raw[64:128, :]
                                )
                                nc.sync.dma_start(
                                    out=swp[64:128, :], in_=raw[0:64, :]
                                )
                                t1 = ropep.tile([128, TC], f32, tag="t1")
                                nc.vector.tensor_mul(
                                    t1, raw, cc_sb[:, soff : soff + TC]
                                )
                                nc.vector.tensor_mul(
                                    swp, swp, ss_sb[:, soff : soff + TC]
                                )
                                nc.vector.tensor_add(dst, t1, swp)
                        for tt in range(TC // 128):
                            psv = ps1.tile([128, OC], f32, tag="ps_v")
                            for kk in range(KT):
                                nc.tensor.matmul(
                                    psv,
                                    hch[:, kk, tt * 128 : (tt + 1) * 128],
                                    wv_sb[:, kk, :],
                                    start=(kk == 0),
                                    stop=(kk == KT - 1),
                                )
                            nc.scalar.copy(
                                v_sb[:, tci * (TC // 128) + tt, :], psv
                            )

                # ---- phase 2: attention per head ----
                outT_sb = qkvp.tile([128, HPC, S], fr, tag="outT")
                with (
                    tc.tile_pool(
                        name="ps2s", bufs=(2 if R_EXP2 else 4), space="PSUM"
                    ) as ps2s,
                    tc.tile_pool(name="ps2o", bufs=2, space="PSUM") as ps2o,
                    tc.tile_pool(name="ps2d", bufs=2, space="PSUM") as ps2d,
                ):
                    for hl in range(HPC):
                        qTap = qk_sb[:, hl, :]
                        kTap = qk_sb[:, 2 + hl, :]
                        for qci in range(S // QC):
                            q0 = qci * QC
                            psO = ps2o.tile([128, QC], f32, tag="psO")
                            psD = ps2d.tile([128, QC], f32, tag="psD")
                            nkt = S // 128
                            for kg in range(nkt // 2):
                                if R_EXP2:
                                    # two score tiles into one 2-bank PSUM
                                    # tile, one Exp drain over both banks
                                    psS = ps2s.tile([128, 2, QC], f32, tag="psS")
                                    for j in range(2):
                                        kt = kg * 2 + j
                                        nc.tensor.matmul(
                                            psS[:, j, :],
                                            kTap[:, kt * 128 : (kt + 1) * 128],
                                            qTap[:, q0 : q0 + QC],
                                            skip_group_check=True,
                                        )
                                    pe = pp.tile([128, 2, QC], fr, tag="pexp")
                                    nc.scalar.activation(pe, psS, Exp, scale=SCALE)
                                else:
                                    pe = pp.tile([128, 2, QC], fr, tag="pexp")
                                    for j in range(2):
                                        kt = kg * 2 + j
                                        psS = ps2s.tile(
                                            [128, QC], f32, tag="psS"
                                        )
                                        nc.tensor.matmul(
                                            psS,
                                            kTap[:, kt * 128 : (kt + 1) * 128],
                                            qTap[:, q0 : q0 + QC],
                                            skip_group_check=True,
                                        )
                                        nc.scalar.activation(
                                            pe[:, j, :], psS, Exp, scale=SCALE
                                        )
                                for j in range(2):
                                    kt = kg * 2 + j
                                    first = kt == 0
                                    last = kt == nkt - 1
                                    nc.tensor.matmul(
                                        psO,
                                        v_sb[:, kt, hl * DH : (hl + 1) * DH],
                                        pe[:, j, :],
                                        start=first,
                                        stop=last,
                                        skip_group_check=True,
                                    )
                                    nc.tensor.matmul(
                                        psD,
                                        ones_sb,
                                        pe[:, j, :],
                                        start=first,
                                        stop=last,
                                        skip_group_check=True,
                                    )
                            rd = smallp.tile([128, QC], f32, tag="rd")
                            if R_RECIP:
                                nc.vector.reciprocal_approx_fast(rd, psD)
                            else:
                                nc.vector.reciprocal(rd, psD)
                            nc.vector.tensor_mul(
                                outT_sb[:, hl, q0 : q0 + QC], psO, rd
                            )

                # ---- phase 3: output projection (partial over this core's heads) ----
                with tc.tile_pool(name="ps3", bufs=4, space="PSUM") as ps3:
                    for tt in range(S // 128):
                        for hw in range(2):
                            fo = foutp.tile([128, HID // 2], bf16, tag="fo")
                            for nhs in range(2):
                                nh = hw * 2 + nhs
                                psF = ps3.tile([128, 512], f32, tag="psF")
                                for hl in range(HPC):
                                    nc.tensor.matmul(
                                        psF,
                                        outT_sb[:, hl, tt * 128 : (tt + 1) * 128],
                                        wo_sb[:, hl, nh * 512 : (nh + 1) * 512],
                                        start=(hl == 0),
                                        stop=(hl == HPC - 1),
                                    )
                                nc.vector.tensor_copy(
                                    fo[:, nhs * 512 : (nhs + 1) * 512], psF
                                )
                            nc.sync.dma_start(
                                out=out_p[
                                    t0 + tt * 128 : t0 + (tt + 1) * 128,
                                    hw * 1024 : (hw + 1) * 1024,
                                ],
                                in_=fo,
                            )

    nc.compile()
    return nc


def _deint(idx128):
    """de-interleave a [128] index block: evens then odds."""
    return np.concatenate([idx128[0::2], idx128[1::2]])


def _prep_inputs(hidden_states, cos, sin, w_qkv, w_o):
    """Host-side shard/layout prep. Returns per-core input maps."""
    hs = np.ascontiguousarray(
        hidden_states.reshape(T, HID).T, dtype=np.float32
    )  # [HID, T]
    ccf = np.ascontiguousarray(
        np.concatenate([cos.T[0::2, :], cos.T[1::2, :]], axis=0), dtype=np.float32
    )  # [128, S] de-interleaved
    ssf = np.ascontiguousarray(
        np.concatenate([-sin.T[0::2, :], sin.T[1::2, :]], axis=0), dtype=np.float32
    )  # [128, S] de-interleaved, sign folded

    in_maps = []
    for c in range(NC):
        heads = [HPC * c + i for i in range(HPC)]
        qrows = np.concatenate([_deint(np.arange(h * DH, (h + 1) * DH)) for h in heads])
        krows = H * DH + qrows
        vrows = (
            np.concatenate([np.arange(h * DH, (h + 1) * DH) for h in heads])
            + 2 * H * DH
        )
        ocols = np.concatenate([np.arange(h * DH, (h + 1) * DH) for h in heads])
        in_maps.append(
            {
                "hT": hs,
                "wqT": np.ascontiguousarray(w_qkv[qrows, :].T, dtype=np.float32),
                "wkT": np.ascontiguousarray(w_qkv[krows, :].T, dtype=np.float32),
                "wvT": np.ascontiguousarray(w_qkv[vrows, :].T, dtype=np.float32),
                "woT": np.ascontiguousarray(w_o[:, ocols].T, dtype=np.float32),
                "cc": ccf,
                "ss": ssf,
            }
        )
    return in_maps


def kernel(hidden_states, cos, sin, w_qkv, w_o):
    global _exec_time_ns
    from concourse.bass_utils import run_bass_kernel_spmd

    hidden_states = np.asarray(hidden_states, dtype=np.float32)
    cos = np.asarray(cos, dtype=np.float32)
    sin = np.asarray(sin, dtype=np.float32)
    w_qkv = np.asarray(w_qkv, dtype=np.float32)
    w_o = np.asarray(w_o, dtype=np.float32)

    nc = _build()
    in_maps = _prep_inputs(hidden_states, cos, sin, w_qkv, w_o)
    res = run_bass_kernel_spmd(
        nc,
        in_maps,
        core_ids=list(range(NC)),
        trace=bool(int(os.environ.get("KERNEL_TRACE", "0"))),
    )
    _exec_time_ns = res.exec_time_ns
    globals()["_last_result"] = res

    acc = res.results[0]["out_p"].astype(np.float32)
    for c in range(1, NC):
        acc = acc + res.results[c]["out_p"].astype(np.float32)
    return acc.reshape(B, S, HID)


# revision 20
# speedup vs baseline: 1.1034x; 1.1034x over previous
"""Trainium2 Bass kernel for a full attention layer (QKV proj + interleaved
RoPE + non-causal SDPA + output proj), tensor-parallel over heads on 8
NeuronCores.

Hardcoded problem shape: B=2, S=2048, HID=2048, H=16 heads, DH=128, fp32.

Sharding (per core c of 8): heads 2c, 2c+1.
 - w_qkv rows for those heads (q/k rows de-interleaved per head so RoPE's
   (2i, 2i+1) pairing becomes a 64-partition block swap), transposed to
   [HID, 256] so the contraction dim (HID) rides the SBUF partition axis.
 - w_o columns for those heads, transposed to [256, HID].
 - hidden_states transposed to [HID, B*S] (replicated to every core).
 - cos/sin prepped as de-interleaved, transposed [128, S] tiles; sin carries
   the rotate-half sign in its first 64 rows.
Each core computes a full-shape partial output [B*S, HID] (its heads'
contribution through w_o) in bf16; the host unshards by summing the 8
partials in fp32.

All matmuls run as float32r (full PE rate for moving dim >= 256; fp32 data).
Attention is computed in the S^T orientation: scores come out as
P^T[k, q] tiles so the AV matmul can contract k on the partition axis with
no transposes anywhere.  The softmax denominator is an all-ones [128,128]
stationary matmul accumulated alongside AV; out tiles are scaled by its
approx reciprocal after AV (divide-after-AV).  exp() is fused into the
PSUM->SBUF drain on the scalar engine over two PSUM banks at a time, with
the 1/sqrt(DH) scale folded in.  No max-subtraction: scores are ~N(0,1) so
exp is safe in fp32.  RoPE runs on the vector engine directly out of PSUM
using cross-partition-offset operands (no swap DMAs, no scalar copies).
DMA dispatch is split across engine queues: weights on the scalar queue,
cos/sin on gpsimd, activations + output stores on sync.
"""

import os

import numpy as np

B, S, HID = 2, 2048, 2048
H, DH = 16, 128
NC = 8
HPC = H // NC          # heads per core = 2
OC = HPC * DH          # per-core o width per section = 256
T = B * S              # 4096 tokens
KT = HID // 128        # 16 contraction tiles
TC = 256               # token chunk for QKV projection
QC = 512               # query chunk for attention
SCALE = 1.0 / float(np.sqrt(DH))

_exec_time_ns = None   # stashed by kernel() for the test harness


R_EXP2 = bool(int(os.environ.get("R_EXP2", "0")))    # Exp over 2-bank PSUM span
R_ROPE = bool(int(os.environ.get("R_ROPE", "0")))    # DVE cross-partition RoPE from PSUM
R_RECIP = bool(int(os.environ.get("R_RECIP", "1")))  # reciprocal_approx_fast
R_DMAQ = bool(int(os.environ.get("R_DMAQ", "1")))    # split DMA dispatch queues
R_ILV = bool(int(os.environ.get("R_ILV", "0")))      # interleave o-proj into attention


def _build():
    import concourse.bacc as bacc
    import concourse.mybir as mybir
    import concourse.tile as tile

    f32 = mybir.dt.float32
    fr = mybir.dt.float32r
    bf16 = mybir.dt.bfloat16
    Exp = mybir.ActivationFunctionType.Exp

    nc = bacc.Bacc("TRN2", target_bir_lowering=False)

    hT = nc.dram_tensor("hT", [HID, T], fr, kind="ExternalInput")
    wqT = nc.dram_tensor("wqT", [HID, OC], fr, kind="ExternalInput")
    wkT = nc.dram_tensor("wkT", [HID, OC], fr, kind="ExternalInput")
    wvT = nc.dram_tensor("wvT", [HID, OC], fr, kind="ExternalInput")
    woT = nc.dram_tensor("woT", [OC, HID], fr, kind="ExternalInput")
    cc = nc.dram_tensor("cc", [DH, S], f32, kind="ExternalInput")
    ss = nc.dram_tensor("ss", [DH, S], f32, kind="ExternalInput")
    out_p = nc.dram_tensor("out_p", [T, HID], bf16, kind="ExternalOutput")

    hT_r = hT.rearrange("(k p) t -> p k t", p=128)      # [128, 16, T]
    wqT_r = wqT.rearrange("(k p) o -> p k o", p=128)    # [128, 16, 256]
    wkT_r = wkT.rearrange("(k p) o -> p k o", p=128)
    wvT_r = wvT.rearrange("(k p) o -> p k o", p=128)
    woT_r = woT.rearrange("(h p) n -> p h n", p=128)    # [128, 2, 2048]

    with tile.TileContext(nc) as tc:
        with (
            tc.tile_pool(name="const", bufs=1) as constp,
            tc.tile_pool(name="hbuf", bufs=2) as hpool,
            tc.tile_pool(name="qkv", bufs=1) as qkvp,
            tc.tile_pool(name="rope", bufs=2) as ropep,
            tc.tile_pool(name="pbuf", bufs=3) as pp,
            tc.tile_pool(name="small", bufs=2) as smallp,
            tc.tile_pool(name="fout", bufs=2) as foutp,
        ):
            # ---- resident weights/constants: one DMA each, on side queues ----
            # weights fan out across queues so the first qk matmul's deps
            # (wq + first hch chunk) land as early as possible
            wq_sb = constp.tile([128, KT, OC], fr)
            wk_sb = constp.tile([128, KT, OC], fr)
            wv_sb = constp.tile([128, KT, OC], fr)
            nc.scalar.dma_start(out=wq_sb, in_=wqT_r)
            nc.gpsimd.dma_start(out=wk_sb, in_=wkT_r)
            nc.gpsimd.dma_start(out=wv_sb, in_=wvT_r)
            wo_sb = constp.tile([128, HPC, HID], fr)
            nc.scalar.dma_start(out=wo_sb, in_=woT_r)
            cc_sb = constp.tile([128, S], f32)
            ss_sb = constp.tile([128, S], f32)
            nc.scalar.dma_start(out=cc_sb, in_=cc[:, :])
            nc.gpsimd.dma_start(out=ss_sb, in_=ss[:, :])
            ones_f32 = constp.tile([128, 128], f32)
            nc.vector.memset(ones_f32, 1.0)
            ones_sb = constp.tile([128, 128], fr)
            nc.vector.tensor_copy(ones_sb, ones_f32)

            w_of = [(wq_sb, 0), (wq_sb, 1), (wk_sb, 0), (wk_sb, 1)]

            def phase23_ilv(t0, qk_sb, v_sb, outT_sb):
                """Attention with phase-3 o-proj groups of the previous
                query chunk interleaved into the kg loop: the o-proj
                matmuls fill PE slack while the scalar engine works
                through the Exp drains."""
                nkt = S // 128
                fo_cur = [None]

                with (
                    tc.tile_pool(name="ps2s", bufs=2, space="PSUM") as ps2s,
                    tc.tile_pool(name="ps2o", bufs=2, space="PSUM") as ps2o,
                    tc.tile_pool(name="ps2d", bufs=2, space="PSUM") as ps2d,
                    tc.tile_pool(name="ps3i", bufs=2, space="PSUM") as ps3,
                ):
                    def ph3_group(qprev, g):
                        # g in [0,16): (tt4, hw, nhs) o-proj unit of 2 mm
                        tt = qprev * 4 + g // 4
                        hw = (g % 4) // 2
                        nhs = g % 2
                        nh = hw * 2 + nhs
                        if nhs == 0:
                            fo_cur[0] = foutp.tile(
                                [128, HID // 2], bf16, tag="fo", name="fo_i"
                            )
                        psF = ps3.tile([128, 512], f32, tag="psF")
                        for hl in range(HPC):
                            nc.tensor.matmul(
                                psF,
                                outT_sb[:, hl, tt * 128 : (tt + 1) * 128],
                                wo_sb[:, hl, nh * 512 : (nh + 1) * 512],
                                start=(hl == 0),
                                stop=(hl == HPC - 1),
                            )
                        nc.vector.tensor_copy(
                            fo_cur[0][:, nhs * 512 : (nhs + 1) * 512], psF
                        )
                        if nhs == 1:
                            nc.sync.dma_start(
                                out=out_p[
                                    t0 + tt * 128 : t0 + (tt + 1) * 128,
                                    hw * 1024 : (hw + 1) * 1024,
                                ],
                                in_=fo_cur[0],
                            )

                    for qci in range(S // QC):
                        q0 = qci * QC
                        for hl in range(HPC):
                            qTap = qk_sb[:, hl, :]
                            kTap = qk_sb[:, 2 + hl, :]
                            psO = ps2o.tile([128, QC], f32, tag="psO")
                            psD = ps2d.tile([128, QC], f32, tag="psD")
                            for kg in range(nkt // 2):
                                pe = pp.tile([128, 2, QC], fr, tag="pexp")
                                for j in range(2):
                                    kt = kg * 2 + j
                                    psS = ps2s.tile([128, QC], f32, tag="psS")
                                    nc.tensor.matmul(
                                        psS,
                                        kTap[:, kt * 128 : (kt + 1) * 128],
                                        qTap[:, q0 : q0 + QC],
                                        skip_group_check=True,
                                    )
                                    nc.scalar.activation(
                                        pe[:, j, :], psS, Exp, scale=SCALE
                                    )
                                for j in range(2):
                                    kt = kg * 2 + j
                                    first = kt == 0
                                    last = kt == nkt - 1
                                    nc.tensor.matmul(
                                        psO,
                                        v_sb[:, kt, hl * DH : (hl + 1) * DH],
                                        pe[:, j, :],
                                        start=first,
                                        stop=last,
                                        skip_group_check=True,
                                    )
                                    nc.tensor.matmul(
                                        psD,
                                        ones_sb,
                                        pe[:, j, :],
                                        start=first,
                                        stop=last,
                                        skip_group_check=True,
                                    )
                                if qci > 0:
                                    ph3_group(qci - 1, hl * 8 + kg)
                            rd = smallp.tile([128, QC], f32, tag="rd")
                            nc.vector.reciprocal_approx_fast(rd, psD)
                            nc.vector.tensor_mul(
                                outT_sb[:, hl, q0 : q0 + QC], psO, rd
                            )
                    for g in range(16):
                        ph3_group(S // QC - 1, g)

            for b in range(B):
                t0 = b * S

                # ---- phase 1: QKV projection (+ fused RoPE for q,k) ----
                # qk_sb rows: [q_h0, q_h1, k_h0, k_h1], each [128 d, S]
                qk_sb = qkvp.tile([128, 4, S], fr, tag="qk")
                v_sb = qkvp.tile([128, KT, OC], fr, tag="v")
                with tc.tile_pool(name="ps1", bufs=4, space="PSUM") as ps1:
                    for tci in range(S // TC):
                        soff = tci * TC
                        hch = hpool.tile([128, KT, TC], fr, tag="hch")
                        nc.sync.dma_start(
                            out=hch, in_=hT_r[:, :, t0 + soff : t0 + soff + TC]
                        )
                        for ot in range(4):
                            wsb, hl = w_of[ot]
                            ps = ps1.tile([128, TC], f32, tag="ps_qk")
                            for kk in range(KT):
                                nc.tensor.matmul(
                                    ps,
                                    wsb[:, kk, hl * DH : (hl + 1) * DH],
                                    hch[:, kk, :],
                                    start=(kk == 0),
                                    stop=(kk == KT - 1),
                                )
                            # RoPE: dst = ps*cc + blockswap(ps)*ss_signed
                            dst = qk_sb[:, ot, soff : soff + TC]
                            if R_ROPE:
                                # DVE reads PSUM directly, cross-partition
                                t2 = ropep.tile([128, TC], f32, tag="t2")
                                nc.vector.tensor_mul(
                                    t2[0:64, :],
                                    ps[64:128, :],
                                    ss_sb[0:64, soff : soff + TC],
                                )
                                nc.vector.tensor_mul(
                                    t2[64:128, :],
                                    ps[0:64, :],
                                    ss_sb[64:128, soff : soff + TC],
                                )
                                t1 = ropep.tile([128, TC], f32, tag="t1")
                                nc.vector.tensor_mul(
                                    t1, ps, cc_sb[:, soff : soff + TC]
                                )
                                nc.vector.tensor_add(dst, t1, t2)
                            else:
                                raw = ropep.tile([128, TC], f32, tag="raw")
                                nc.scalar.copy(raw, ps)
                                swp = ropep.tile([128, TC], f32, tag="swp")
                                # swaps ride the idle gpsimd queue so they
                                # don't delay hch prefetches on sync
                                nc.gpsimd.dma_start(
                                    out=swp[0:64, :], in_=raw[64:128, :]
                                )
                                nc.gpsimd.dma_start(
                                    out=swp[64:128, :], in_=raw[0:64, :]
                                )
                                t1 = ropep.tile([128, TC], f32, tag="t1")
                                nc.vector.tensor_mul(
                                    t1, raw, cc_sb[:, soff : soff + TC]
                                )
                                nc.vector.tensor_mul(
                                    swp, swp, ss_sb[:, soff : soff + TC]
                                )
                                nc.vector.tensor_add(dst, t1, swp)
                        for tt in range(TC // 128):
                            psv = ps1.tile([128, OC], f32, tag="ps_v")
                            for kk in range(KT):
                                nc.tensor.matmul(
                                    psv,
                                    hch[:, kk, tt * 128 : (tt + 1) * 128],
                                    wv_sb[:, kk, :],
                                    start=(kk == 0),
                                    stop=(kk == KT - 1),
                                )
                            nc.scalar.copy(
                                v_sb[:, tci * (TC // 128) + tt, :], psv
                            )

                # ---- phase 2: attention per head ----
                outT_sb = qkvp.tile([128, HPC, S], fr, tag="outT")
                if R_ILV:
                    phase23_ilv(t0, qk_sb, v_sb, outT_sb)
                    continue
                with (
                    tc.tile_pool(
                        name="ps2s", bufs=(2 if R_EXP2 else 4), space="PSUM"
                    ) as ps2s,
                    tc.tile_pool(name="ps2o", bufs=2, space="PSUM") as ps2o,
                    tc.tile_pool(name="ps2d", bufs=2, space="PSUM") as ps2d,
                ):
                    for hl in range(HPC):
                        qTap = qk_sb[:, hl, :]
                        kTap = qk_sb[:, 2 + hl, :]
                        for qci in range(S // QC):
                            q0 = qci * QC
                            psO = ps2o.tile([128, QC], f32, tag="psO")
                            psD = ps2d.tile([128, QC], f32, tag="psD")
                            nkt = S // 128
                            for kg in range(nkt // 2):
                                if R_EXP2:
                                    # two score tiles into one 2-bank PSUM
                                    # tile, one Exp drain over both banks
                                    psS = ps2s.tile([128, 2, QC], f32, tag="psS")
                                    for j in range(2):
                                        kt = kg * 2 + j
                                        nc.tensor.matmul(
                                            psS[:, j, :],
                                            kTap[:, kt * 128 : (kt + 1) * 128],
                                            qTap[:, q0 : q0 + QC],
                                            skip_group_check=True,
                                        )
                                    pe = pp.tile([128, 2, QC], fr, tag="pexp")
                                    nc.scalar.activation(pe, psS, Exp, scale=SCALE)
                                else:
                                    pe = pp.tile([128, 2, QC], fr, tag="pexp")
                                    for j in range(2):
                                        kt = kg * 2 + j
                                        psS = ps2s.tile(
                                            [128, QC], f32, tag="psS"
                                        )
                                        nc.tensor.matmul(
                                            psS,
                                            kTap[:, kt * 128 : (kt + 1) * 128],
                                            qTap[:, q0 : q0 + QC],
                                            skip_group_check=True,
                                        )
                                        nc.scalar.activation(
                                            pe[:, j, :], psS, Exp, scale=SCALE
                                        )
                                for j in range(2):
                                    kt = kg * 2 + j
                                    first = kt == 0
                                    last = kt == nkt - 1
                                    nc.tensor.matmul(
                                        psO,
                                        v_sb[:, kt, hl * DH : (hl + 1) * DH],
                                        pe[:, j, :],
                                        start=first,
                                        stop=last,
                                        skip_group_check=True,
                                    )
                                    nc.tensor.matmul(
                                        psD,
                                        ones_sb,
                                        pe[:, j, :],
                                        start=first,
                                        stop=last,
                                        skip_group_check=True,
                                    )
                            rd = smallp.tile([128, QC], f32, tag="rd")
                            if R_RECIP:
                                nc.vector.reciprocal_approx_fast(rd, psD)
                            else:
                                nc.vector.reciprocal(rd, psD)
                            nc.vector.tensor_mul(
                                outT_sb[:, hl, q0 : q0 + QC], psO, rd
                            )

                # ---- phase 3: output projection (partial over this core's heads) ----
                with tc.tile_pool(name="ps3", bufs=4, space="PSUM") as ps3:
                    for tt in range(S // 128):
                        for hw in range(2):
                            fo = foutp.tile([128, HID // 2], bf16, tag="fo")
                            for nhs in range(2):
                                nh = hw * 2 + nhs
                                psF = ps3.tile([128, 512], f32, tag="psF")
                                for hl in range(HPC):
                                    nc.tensor.matmul(
                                        psF,
                                        outT_sb[:, hl, tt * 128 : (tt + 1) * 128],
                                        wo_sb[:, hl, nh * 512 : (nh + 1) * 512],
                                        start=(hl == 0),
                                        stop=(hl == HPC - 1),
                                    )
                                # alternate drains across DVE and ACT
                                if nh % 2 == 0:
                                    nc.vector.tensor_copy(
                                        fo[:, nhs * 512 : (nhs + 1) * 512], psF
                                    )
                                else:
                                    nc.scalar.copy(
                                        fo[:, nhs * 512 : (nhs + 1) * 512], psF
                                    )
                            nc.sync.dma_start(
                                out=out_p[
                                    t0 + tt * 128 : t0 + (tt + 1) * 128,
                                    hw * 1024 : (hw + 1) * 1024,
                                ],
                                in_=fo,
                            )

    nc.compile()
    return nc


def _deint(idx128):
    """de-interleave a [128] index block: evens then odds."""
    return np.concatenate([idx128[0::2], idx128[1::2]])


def _prep_inputs(hidden_states, cos, sin, w_qkv, w_o):
    """Host-side shard/layout prep. Returns per-core input maps."""
    hs = np.ascontiguousarray(
        hidden_states.reshape(T, HID).T, dtype=np.float32
    )  # [HID, T]
    ccf = np.ascontiguousarray(
        np.concatenate([cos.T[0::2, :], cos.T[1::2, :]], axis=0), dtype=np.float32
    )  # [128, S] de-interleaved
    ssf = np.ascontiguousarray(
        np.concatenate([-sin.T[0::2, :], sin.T[1::2, :]], axis=0), dtype=np.float32
    )  # [128, S] de-interleaved, sign folded

    in_maps = []
    for c in range(NC):
        heads = [HPC * c + i for i in range(HPC)]
        qrows = np.concatenate([_deint(np.arange(h * DH, (h + 1) * DH)) for h in heads])
        krows = H * DH + qrows
        vrows = (
            np.concatenate([np.arange(h * DH, (h + 1) * DH) for h in heads])
            + 2 * H * DH
        )
        ocols = np.concatenate([np.arange(h * DH, (h + 1) * DH) for h in heads])
        in_maps.append(
            {
                "hT": hs,
                "wqT": np.ascontiguousarray(w_qkv[qrows, :].T, dtype=np.float32),
                "wkT": np.ascontiguousarray(w_qkv[krows, :].T, dtype=np.float32),
                "wvT": np.ascontiguousarray(w_qkv[vrows, :].T, dtype=np.float32),
                "woT": np.ascontiguousarray(w_o[:, ocols].T, dtype=np.float32),
                "cc": ccf,
                "ss": ssf,
            }
        )
    return in_maps


def kernel(hidden_states, cos, sin, w_qkv, w_o):
    global _exec_time_ns
    from concourse.bass_utils import run_bass_kernel_spmd

    hidden_states = np.asarray(hidden_states, dtype=np.float32)
    cos = np.asarray(cos, dtype=np.float32)
    sin = np.asarray(sin, dtype=np.float32)
    w_qkv = np.asarray(w_qkv, dtype=np.float32)
    w_o = np.asarray(w_o, dtype=np.float32)

    nc = _build()
    in_maps = _prep_inputs(hidden_states, cos, sin, w_qkv, w_o)
    res = run_bass_kernel_spmd(
        nc,
        in_maps,
        core_ids=list(range(NC)),
        trace=bool(int(os.environ.get("KERNEL_TRACE", "0"))),
    )
    _exec_time_ns = res.exec_time_ns
    globals()["_last_result"] = res

    acc = res.results[0]["out_p"].astype(np.float32)
    for c in range(1, NC):
        acc = acc + res.results[c]["out_p"].astype(np.float32)
    return acc.reshape(B, S, HID)


# revision 23
# speedup vs baseline: 1.2197x; 1.1055x over previous
"""Trainium2 Bass kernel for a full attention layer (QKV proj + interleaved
RoPE + non-causal SDPA + output proj), tensor-parallel over heads on 8
NeuronCores.

Hardcoded problem shape: B=2, S=2048, HID=2048, H=16 heads, DH=128, fp32.

Sharding (per core c of 8): heads 2c, 2c+1.
 - w_qkv rows for those heads (q/k rows de-interleaved per head so RoPE's
   (2i, 2i+1) pairing becomes a 64-partition block swap), transposed to
   [HID, 256] so the contraction dim (HID) rides the SBUF partition axis.
 - w_o columns for those heads, transposed to [256, HID].
 - hidden_states transposed to [HID, B*S] (replicated to every core).
 - cos/sin prepped as de-interleaved, transposed [128, S] tiles; sin carries
   the rotate-half sign in its first 64 rows.
Each core computes a full-shape partial output [B*S, HID] (its heads'
contribution through w_o) in bf16; the host unshards by summing the 8
partials in fp32.

All matmuls run as float32r (full PE rate for moving dim >= 256; fp32 data).
Attention is computed in the S^T orientation: scores come out as
P^T[k, q] tiles so the AV matmul can contract k on the partition axis with
no transposes anywhere.  The softmax denominator is an all-ones [128,128]
stationary matmul accumulated alongside AV; out tiles are scaled by its
approx reciprocal after AV (divide-after-AV).  exp() is fused into the
PSUM->SBUF drain on the scalar engine over two PSUM banks at a time, with
the 1/sqrt(DH) scale folded in.  No max-subtraction: scores are ~N(0,1) so
exp is safe in fp32.  RoPE runs on the vector engine directly out of PSUM
using cross-partition-offset operands (no swap DMAs, no scalar copies).
DMA dispatch is split across engine queues: weights on the scalar queue,
cos/sin on gpsimd, activations + output stores on sync.
"""

import os

import numpy as np

B, S, HID = 2, 2048, 2048
H, DH = 16, 128
NC = 8
HPC = H // NC          # heads per core = 2
OC = HPC * DH          # per-core o width per section = 256
T = B * S              # 4096 tokens
KT = HID // 128        # 16 contraction tiles
TC = 256               # token chunk for QKV projection
QC = 512               # query chunk for attention
SCALE = 1.0 / float(np.sqrt(DH))

_exec_time_ns = None   # stashed by kernel() for the test harness


R_EXP2 = bool(int(os.environ.get("R_EXP2", "0")))    # Exp over 2-bank PSUM span
R_ROPE = bool(int(os.environ.get("R_ROPE", "0")))    # DVE cross-partition RoPE from PSUM
R_RECIP = bool(int(os.environ.get("R_RECIP", "1")))  # reciprocal_approx_fast
R_DMAQ = bool(int(os.environ.get("R_DMAQ", "1")))    # split DMA dispatch queues
R_ILV = bool(int(os.environ.get("R_ILV", "0")))      # interleave o-proj into attention


def _build():
    import concourse.bacc as bacc
    import concourse.mybir as mybir
    import concourse.tile as tile

    f32 = mybir.dt.float32
    fr = mybir.dt.float32r
    bf16 = mybir.dt.bfloat16
    Exp = mybir.ActivationFunctionType.Exp

    nc = bacc.Bacc("TRN2", target_bir_lowering=False)

    hT = nc.dram_tensor("hT", [HID, T], fr, kind="ExternalInput")
    wqT = nc.dram_tensor("wqT", [HID, OC], fr, kind="ExternalInput")
    wkT = nc.dram_tensor("wkT", [HID, OC], fr, kind="ExternalInput")
    wvT = nc.dram_tensor("wvT", [HID, OC], fr, kind="ExternalInput")
    woT = nc.dram_tensor("woT", [OC, HID], fr, kind="ExternalInput")
    cc = nc.dram_tensor("cc", [DH, S], f32, kind="ExternalInput")
    ss = nc.dram_tensor("ss", [DH, S], f32, kind="ExternalInput")
    out_p = nc.dram_tensor("out_p", [T, HID], bf16, kind="ExternalOutput")

    hT_r = hT.rearrange("(k p) t -> p k t", p=128)      # [128, 16, T]
    wqT_r = wqT.rearrange("(k p) o -> p k o", p=128)    # [128, 16, 256]
    wkT_r = wkT.rearrange("(k p) o -> p k o", p=128)
    wvT_r = wvT.rearrange("(k p) o -> p k o", p=128)
    woT_r = woT.rearrange("(h p) n -> p h n", p=128)    # [128, 2, 2048]

    with tile.TileContext(nc) as tc:
        with (
            tc.tile_pool(name="const", bufs=1) as constp,
            tc.tile_pool(name="hbuf", bufs=2) as hpool,
            tc.tile_pool(name="qkv", bufs=1) as qkvp,
            tc.tile_pool(name="rope", bufs=2) as ropep,
            tc.tile_pool(name="pbuf", bufs=3) as pp,
            tc.tile_pool(name="small", bufs=2) as smallp,
            tc.tile_pool(name="fout", bufs=2) as foutp,
        ):
            # ---- resident weights/constants: one DMA each, on side queues ----
            # weights fan out across queues so the first qk matmul's deps
            # (wq + first hch chunk) land as early as possible
            wq_sb = constp.tile([128, KT, OC], fr)
            wk_sb = constp.tile([128, KT, OC], fr)
            wv_sb = constp.tile([128, KT, OC], fr)
            nc.scalar.dma_start(out=wq_sb, in_=wqT_r)
            nc.scalar.dma_start(out=wk_sb, in_=wkT_r)
            nc.scalar.dma_start(out=wv_sb, in_=wvT_r)
            wo_sb = constp.tile([128, HPC, HID], fr)
            nc.scalar.dma_start(out=wo_sb, in_=woT_r)
            cc_sb = constp.tile([128, S], f32)
            ss_sb = constp.tile([128, S], f32)
            nc.scalar.dma_start(out=cc_sb, in_=cc[:, :])
            nc.scalar.dma_start(out=ss_sb, in_=ss[:, :])
            ones_f32 = constp.tile([128, 128], f32)
            nc.vector.memset(ones_f32, 1.0)
            ones_sb = constp.tile([128, 128], fr)
            nc.vector.tensor_copy(ones_sb, ones_f32)

            w_of = [(wq_sb, 0), (wq_sb, 1), (wk_sb, 0), (wk_sb, 1)]

            def phase23_ilv(t0, qk_sb, v_sb, outT_sb):
                """Attention with phase-3 o-proj groups of the previous
                query chunk interleaved into the kg loop: the o-proj
                matmuls fill PE slack while the scalar engine works
                through the Exp drains."""
                nkt = S // 128
                fo_cur = [None]

                with (
                    tc.tile_pool(name="ps2s", bufs=2, space="PSUM") as ps2s,
                    tc.tile_pool(name="ps2o", bufs=2, space="PSUM") as ps2o,
                    tc.tile_pool(name="ps2d", bufs=2, space="PSUM") as ps2d,
                    tc.tile_pool(name="ps3i", bufs=2, space="PSUM") as ps3,
                ):
                    def ph3_group(qprev, g):
                        # g in [0,16): (tt4, hw, nhs) o-proj unit of 2 mm
                        tt = qprev * 4 + g // 4
                        hw = (g % 4) // 2
                        nhs = g % 2
                        nh = hw * 2 + nhs
                        if nhs == 0:
                            fo_cur[0] = foutp.tile(
                                [128, HID // 2], bf16, tag="fo", name="fo_i"
                            )
                        psF = ps3.tile([128, 512], f32, tag="psF")
                        for hl in range(HPC):
                            nc.tensor.matmul(
                                psF,
                                outT_sb[:, hl, tt * 128 : (tt + 1) * 128],
                                wo_sb[:, hl, nh * 512 : (nh + 1) * 512],
                                start=(hl == 0),
                                stop=(hl == HPC - 1),
                            )
                        nc.vector.tensor_copy(
                            fo_cur[0][:, nhs * 512 : (nhs + 1) * 512], psF
                        )
                        if nhs == 1:
                            nc.sync.dma_start(
                                out=out_p[
                                    t0 + tt * 128 : t0 + (tt + 1) * 128,
                                    hw * 1024 : (hw + 1) * 1024,
                                ],
                                in_=fo_cur[0],
                            )

                    for qci in range(S // QC):
                        q0 = qci * QC
                        for hl in range(HPC):
                            qTap = qk_sb[:, hl, :]
                            kTap = qk_sb[:, 2 + hl, :]
                            psO = ps2o.tile([128, QC], f32, tag="psO")
                            psD = ps2d.tile([128, QC], f32, tag="psD")
                            for kg in range(nkt // 2):
                                pe = pp.tile([128, 2, QC], fr, tag="pexp")
                                for j in range(2):
                                    kt = kg * 2 + j
                                    psS = ps2s.tile([128, QC], f32, tag="psS")
                                    nc.tensor.matmul(
                                        psS,
                                        kTap[:, kt * 128 : (kt + 1) * 128],
                                        qTap[:, q0 : q0 + QC],
                                        skip_group_check=True,
                                    )
                                    nc.scalar.activation(
                                        pe[:, j, :], psS, Exp, scale=SCALE
                                    )
                                for j in range(2):
                                    kt = kg * 2 + j
                                    first = kt == 0
                                    last = kt == nkt - 1
                                    nc.tensor.matmul(
                                        psO,
                                        v_sb[:, kt, hl * DH : (hl + 1) * DH],
                                        pe[:, j, :],
                                        start=first,
                                        stop=last,
                                        skip_group_check=True,
                                    )
                                    nc.tensor.matmul(
                                        psD,
                                        ones_sb,
                                        pe[:, j, :],
                                        start=first,
                                        stop=last,
                                        skip_group_check=True,
                                    )
                                if qci > 0:
                                    ph3_group(qci - 1, hl * 8 + kg)
                            rd = smallp.tile([128, QC], f32, tag="rd")
                            nc.vector.reciprocal_approx_fast(rd, psD)
                            nc.vector.tensor_mul(
                                outT_sb[:, hl, q0 : q0 + QC], psO, rd
                            )
                    for g in range(16):
                        ph3_group(S // QC - 1, g)

            for b in range(B):
                t0 = b * S

                # ---- phase 1: QKV projection (+ fused RoPE for q,k) ----
                # qk_sb rows: [q_h0, q_h1, k_h0, k_h1], each [128 d, S]
                qk_sb = qkvp.tile([128, 4, S], fr, tag="qk")
                v_sb = qkvp.tile([128, KT, OC], fr, tag="v")
                with tc.tile_pool(name="ps1", bufs=4, space="PSUM") as ps1:
                    for tci in range(S // TC):
                        soff = tci * TC
                        hch = hpool.tile([128, KT, TC], fr, tag="hch")
                        nc.sync.dma_start(
                            out=hch, in_=hT_r[:, :, t0 + soff : t0 + soff + TC]
                        )
                        for ot in range(4):
                            wsb, hl = w_of[ot]
                            ps = ps1.tile([128, TC], f32, tag="ps_qk")
                            for kk in range(KT):
                                nc.tensor.matmul(
                                    ps,
                                    wsb[:, kk, hl * DH : (hl + 1) * DH],
                                    hch[:, kk, :],
                                    start=(kk == 0),
                                    stop=(kk == KT - 1),
                                )
                            # RoPE: dst = ps*cc + blockswap(ps)*ss_signed
                            dst = qk_sb[:, ot, soff : soff + TC]
                            if R_ROPE:
                                # DVE reads PSUM directly, cross-partition
                                t2 = ropep.tile([128, TC], f32, tag="t2")
                                nc.vector.tensor_mul(
                                    t2[0:64, :],
                                    ps[64:128, :],
                                    ss_sb[0:64, soff : soff + TC],
                                )
                                nc.vector.tensor_mul(
                                    t2[64:128, :],
                                    ps[0:64, :],
                                    ss_sb[64:128, soff : soff + TC],
                                )
                                t1 = ropep.tile([128, TC], f32, tag="t1")
                                nc.vector.tensor_mul(
                                    t1, ps, cc_sb[:, soff : soff + TC]
                                )
                                nc.vector.tensor_add(dst, t1, t2)
                            else:
                                raw = ropep.tile([128, TC], f32, tag="raw")
                                nc.scalar.copy(raw, ps)
                                swp = ropep.tile([128, TC], f32, tag="swp")
                                nc.sync.dma_start(
                                    out=swp[0:64, :], in_=raw[64:128, :]
                                )
                                nc.sync.dma_start(
                                    out=swp[64:128, :], in_=raw[0:64, :]
                                )
                                t1 = ropep.tile([128, TC], f32, tag="t1")
                                nc.vector.tensor_mul(
                                    t1, raw, cc_sb[:, soff : soff + TC]
                                )
                                nc.vector.tensor_mul(
                                    swp, swp, ss_sb[:, soff : soff + TC]
                                )
                                nc.vector.tensor_add(dst, t1, swp)
                        for tt in range(TC // 128):
                            psv = ps1.tile([128, OC], f32, tag="ps_v")
                            for kk in range(KT):
                                nc.tensor.matmul(
                                    psv,
                                    hch[:, kk, tt * 128 : (tt + 1) * 128],
                                    wv_sb[:, kk, :],
                                    start=(kk == 0),
                                    stop=(kk == KT - 1),
                                )
                            nc.scalar.copy(
                                v_sb[:, tci * (TC // 128) + tt, :], psv
                            )

                # ---- phase 2: attention per head ----
                outT_sb = qkvp.tile([128, HPC, S], fr, tag="outT")
                if R_ILV:
                    phase23_ilv(t0, qk_sb, v_sb, outT_sb)
                    continue
                with (
                    tc.tile_pool(
                        name="ps2s", bufs=(2 if R_EXP2 else 4), space="PSUM"
                    ) as ps2s,
                    tc.tile_pool(name="ps2o", bufs=2, space="PSUM") as ps2o,
                    tc.tile_pool(name="ps2d", bufs=2, space="PSUM") as ps2d,
                ):
                    for hl in range(HPC):
                        qTap = qk_sb[:, hl, :]
                        kTap = qk_sb[:, 2 + hl, :]
                        for qci in range(S // QC):
                            q0 = qci * QC
                            psO = ps2o.tile([128, QC], f32, tag="psO")
                            psD = ps2d.tile([128, QC], f32, tag="psD")
                            nkt = S // 128
                            for kg in range(nkt // 2):
                                if R_EXP2:
                                    # two score tiles into one 2-bank PSUM
                                    # tile, one Exp drain over both banks
                                    psS = ps2s.tile([128, 2, QC], f32, tag="psS")
                                    for j in range(2):
                                        kt = kg * 2 + j
                                        nc.tensor.matmul(
                                            psS[:, j, :],
                                            kTap[:, kt * 128 : (kt + 1) * 128],
                                            qTap[:, q0 : q0 + QC],
                                            skip_group_check=True,
                                        )
                                    pe = pp.tile([128, 2, QC], fr, tag="pexp")
                                    nc.scalar.activation(pe, psS, Exp, scale=SCALE)
                                else:
                                    pe = pp.tile([128, 2, QC], fr, tag="pexp")
                                    for j in range(2):
                                        kt = kg * 2 + j
                                        psS = ps2s.tile(
                                            [128, QC], f32, tag="psS"
                                        )
                                        nc.tensor.matmul(
                                            psS,
                                            kTap[:, kt * 128 : (kt + 1) * 128],
                                            qTap[:, q0 : q0 + QC],
                                            skip_group_check=True,
                                        )
                                        nc.scalar.activation(
                                            pe[:, j, :], psS, Exp, scale=SCALE
                                        )
                                for j in range(2):
                                    kt = kg * 2 + j
                                    first = kt == 0
                                    last = kt == nkt - 1
                                    nc.tensor.matmul(
                                        psO,
                                        v_sb[:, kt, hl * DH : (hl + 1) * DH],
                                        pe[:, j, :],
                                        start=first,
                                        stop=last,
                                        skip_group_check=True,
                                    )
                                    nc.tensor.matmul(
                                        psD,
                                        ones_sb,
                                        pe[:, j, :],
                                        start=first,
                                        stop=last,
                                        skip_group_check=True,
                                    )
                            rd = smallp.tile([128, QC], f32, tag="rd")
                            if R_RECIP:
                                nc.vector.reciprocal_approx_fast(rd, psD)
                            else:
                                nc.vector.reciprocal(rd, psD)
                            nc.vector.tensor_mul(
                                outT_sb[:, hl, q0 : q0 + QC], psO, rd
                            )

                # ---- phase 3: output projection (partial over this core's heads) ----
                with tc.tile_pool(name="ps3", bufs=4, space="PSUM") as ps3:
                    for tt in range(S // 128):
                        for hw in range(2):
                            fo = foutp.tile([128, HID // 2], bf16, tag="fo")
                            for nhs in range(2):
                                nh = hw * 2 + nhs
                                psF = ps3.tile([128, 512], f32, tag="psF")
                                for hl in range(HPC):
                                    nc.tensor.matmul(
                                        psF,
                                        outT_sb[:, hl, tt * 128 : (tt + 1) * 128],
                                        wo_sb[:, hl, nh * 512 : (nh + 1) * 512],
                                        start=(hl == 0),
                                        stop=(hl == HPC - 1),
                                    )
                                nc.vector.tensor_copy(
                                    fo[:, nhs * 512 : (nhs + 1) * 512], psF
                                )
                            nc.sync.dma_start(
                                out=out_p[
                                    t0 + tt * 128 : t0 + (tt + 1) * 128,
                                    hw * 1024 : (hw + 1) * 1024,
                                ],
                                in_=fo,
                            )

    nc.compile()
    return nc


def _deint(idx128):
    """de-interleave a [128] index block: evens then odds."""
    return np.concatenate([idx128[0::2], idx128[1::2]])


def _prep_inputs(hidden_states, cos, sin, w_qkv, w_o):
    """Host-side shard/layout prep. Returns per-core input maps."""
    hs = np.ascontiguousarray(
        hidden_states.reshape(T, HID).T, dtype=np.float32
    )  # [HID, T]
    ccf = np.ascontiguousarray(
        np.concatenate([cos.T[0::2, :], cos.T[1::2, :]], axis=0), dtype=np.float32
    )  # [128, S] de-interleaved
    ssf = np.ascontiguousarray(
        np.concatenate([-sin.T[0::2, :], sin.T[1::2, :]], axis=0), dtype=np.float32
    )  # [128, S] de-interleaved, sign folded

    in_maps = []
    for c in range(NC):
        heads = [HPC * c + i for i in range(HPC)]
        qrows = np.concatenate([_deint(np.arange(h * DH, (h + 1) * DH)) for h in heads])
        krows = H * DH + qrows
        vrows = (
            np.concatenate([np.arange(h * DH, (h + 1) * DH) for h in heads])
            + 2 * H * DH
        )
        ocols = np.concatenate([np.arange(h * DH, (h + 1) * DH) for h in heads])
        in_maps.append(
            {
                "hT": hs,
                "wqT": np.ascontiguousarray(w_qkv[qrows, :].T, dtype=np.float32),
                "wkT": np.ascontiguousarray(w_qkv[krows, :].T, dtype=np.float32),
                "wvT": np.ascontiguousarray(w_qkv[vrows, :].T, dtype=np.float32),
                "woT": np.ascontiguousarray(w_o[:, ocols].T, dtype=np.float32),
                "cc": ccf,
                "ss": ssf,
            }
        )
    return in_maps


def kernel(hidden_states, cos, sin, w_qkv, w_o):
    global _exec_time_ns
    from concourse.bass_utils import run_bass_kernel_spmd

    hidden_states = np.asarray(hidden_states, dtype=np.float32)
    cos = np.asarray(cos, dtype=np.float32)
    sin = np.asarray(sin, dtype=np.float32)
    w_qkv = np.asarray(w_qkv, dtype=np.float32)
    w_o = np.asarray(w_o, dtype=np.float32)

    nc = _build()
    in_maps = _prep_inputs(hidden_states, cos, sin, w_qkv, w_o)
    res = run_bass_kernel_spmd(
        nc,
        in_maps,
        core_ids=list(range(NC)),
        trace=bool(int(os.environ.get("KERNEL_TRACE", "0"))),
    )
    _exec_time_ns = res.exec_time_ns
    globals()["_last_result"] = res

    acc = res.results[0]["out_p"].astype(np.float32)
    for c in range(1, NC):
        acc = acc + res.results[c]["out_p"].astype(np.float32)
    return acc.reshape(B, S, HID)


# revision 24
# speedup vs baseline: 1.2352x; 1.0126x over previous
"""Trainium2 Bass kernel for a full attention layer (QKV proj + interleaved
RoPE + non-causal SDPA + output proj), tensor-parallel over heads on 8
NeuronCores.

Hardcoded problem shape: B=2, S=2048, HID=2048, H=16 heads, DH=128, fp32.

Sharding (per core c of 8): heads 2c, 2c+1.
 - w_qkv rows for those heads (q/k rows de-interleaved per head so RoPE's
   (2i, 2i+1) pairing becomes a 64-partition block swap), transposed to
   [HID, 256] so the contraction dim (HID) rides the SBUF partition axis.
 - w_o columns for those heads, transposed to [256, HID].
 - hidden_states transposed to [HID, B*S] (replicated to every core).
 - cos/sin prepped as de-interleaved, transposed [128, S] tiles; sin carries
   the rotate-half sign in its first 64 rows.
Each core computes a full-shape partial output [B*S, HID] (its heads'
contribution through w_o) in bf16; the host unshards by summing the 8
partials in fp32.

All matmuls run as float32r (full PE rate for moving dim >= 256; fp32 data).
Attention is computed in the S^T orientation: scores come out as
P^T[k, q] tiles so the AV matmul can contract k on the partition axis with
no transposes anywhere.  The softmax denominator is an all-ones [128,128]
stationary matmul accumulated alongside AV; out tiles are scaled by its
approx reciprocal after AV (divide-after-AV).  exp() is fused into the
PSUM->SBUF drain on the scalar engine over two PSUM banks at a time, with
the 1/sqrt(DH) scale folded in.  No max-subtraction: scores are ~N(0,1) so
exp is safe in fp32.  RoPE runs on the vector engine directly out of PSUM
using cross-partition-offset operands (no swap DMAs, no scalar copies).
DMA dispatch is split across engine queues: weights on the scalar queue,
cos/sin on gpsimd, activations + output stores on sync.
"""

import os

import numpy as np

B, S, HID = 2, 2048, 2048
H, DH = 16, 128
NC = 8
HPC = H // NC          # heads per core = 2
OC = HPC * DH          # per-core o width per section = 256
T = B * S              # 4096 tokens
KT = HID // 128        # 16 contraction tiles
TC = 256               # token chunk for QKV projection
QC = 512               # query chunk for attention
SCALE = 1.0 / float(np.sqrt(DH))

_exec_time_ns = None   # stashed by kernel() for the test harness


R_EXP2 = bool(int(os.environ.get("R_EXP2", "0")))    # Exp over 2-bank PSUM span
R_ROPE = bool(int(os.environ.get("R_ROPE", "0")))    # DVE cross-partition RoPE from PSUM
R_RECIP = bool(int(os.environ.get("R_RECIP", "1")))  # reciprocal_approx_fast
R_DMAQ = bool(int(os.environ.get("R_DMAQ", "1")))    # split DMA dispatch queues
R_ILV = bool(int(os.environ.get("R_ILV", "0")))      # interleave o-proj into attention


def _build():
    import concourse.bacc as bacc
    import concourse.mybir as mybir
    import concourse.tile as tile

    f32 = mybir.dt.float32
    fr = mybir.dt.float32r
    bf16 = mybir.dt.bfloat16
    Exp = mybir.ActivationFunctionType.Exp

    nc = bacc.Bacc("TRN2", target_bir_lowering=False)

    hT = nc.dram_tensor("hT", [HID, T], fr, kind="ExternalInput")
    wqT = nc.dram_tensor("wqT", [HID, OC], fr, kind="ExternalInput")
    wkT = nc.dram_tensor("wkT", [HID, OC], fr, kind="ExternalInput")
    wvT = nc.dram_tensor("wvT", [HID, OC], fr, kind="ExternalInput")
    woT = nc.dram_tensor("woT", [OC, HID], fr, kind="ExternalInput")
    cc = nc.dram_tensor("cc", [DH, S], f32, kind="ExternalInput")
    ss = nc.dram_tensor("ss", [DH, S], f32, kind="ExternalInput")
    out_p = nc.dram_tensor("out_p", [T, HID], bf16, kind="ExternalOutput")

    hT_r = hT.rearrange("(k p) t -> p k t", p=128)      # [128, 16, T]
    wqT_r = wqT.rearrange("(k p) o -> p k o", p=128)    # [128, 16, 256]
    wkT_r = wkT.rearrange("(k p) o -> p k o", p=128)
    wvT_r = wvT.rearrange("(k p) o -> p k o", p=128)
    woT_r = woT.rearrange("(h p) n -> p h n", p=128)    # [128, 2, 2048]

    with tile.TileContext(nc) as tc:
        with (
            tc.tile_pool(name="const", bufs=1) as constp,
            tc.tile_pool(name="hbuf", bufs=2) as hpool,
            tc.tile_pool(name="qkv", bufs=1) as qkvp,
            tc.tile_pool(name="rope", bufs=2) as ropep,
            tc.tile_pool(name="pbuf", bufs=3) as pp,
            tc.tile_pool(name="small", bufs=2) as smallp,
            tc.tile_pool(name="fout", bufs=2) as foutp,
        ):
            # ---- resident weights/constants: one DMA each, on side queues ----
            # weights fan out across queues so the first qk matmul's deps
            # (wq + first hch chunk) land as early as possible
            wq_sb = constp.tile([128, KT, OC], fr)
            wk_sb = constp.tile([128, KT, OC], fr)
            wv_sb = constp.tile([128, KT, OC], fr)
            nc.scalar.dma_start(out=wq_sb, in_=wqT_r)
            nc.scalar.dma_start(out=wk_sb, in_=wkT_r)
            nc.scalar.dma_start(out=wv_sb, in_=wvT_r)
            wo_sb = constp.tile([128, HPC, HID], fr)
            nc.scalar.dma_start(out=wo_sb, in_=woT_r)
            cc_sb = constp.tile([128, S], f32)
            ss_sb = constp.tile([128, S], f32)
            nc.scalar.dma_start(out=cc_sb, in_=cc[:, :])
            nc.scalar.dma_start(out=ss_sb, in_=ss[:, :])
            ones_f32 = constp.tile([128, 128], f32)
            nc.vector.memset(ones_f32, 1.0)
            ones_sb = constp.tile([128, 128], fr)
            nc.vector.tensor_copy(ones_sb, ones_f32)

            w_of = [(wq_sb, 0), (wq_sb, 1), (wk_sb, 0), (wk_sb, 1)]

            def phase23_ilv(t0, qk_sb, v_sb, outT_sb):
                """Attention with phase-3 o-proj groups of the previous
                query chunk interleaved into the kg loop: the o-proj
                matmuls fill PE slack while the scalar engine works
                through the Exp drains."""
                nkt = S // 128
                fo_cur = [None]

                with (
                    tc.tile_pool(name="ps2s", bufs=2, space="PSUM") as ps2s,
                    tc.tile_pool(name="ps2o", bufs=2, space="PSUM") as ps2o,
                    tc.tile_pool(name="ps2d", bufs=2, space="PSUM") as ps2d,
                    tc.tile_pool(name="ps3i", bufs=2, space="PSUM") as ps3,
                ):
                    def ph3_group(qprev, g):
                        # g in [0,16): (tt4, hw, nhs) o-proj unit of 2 mm
                        tt = qprev * 4 + g // 4
                        hw = (g % 4) // 2
                        nhs = g % 2
                        nh = hw * 2 + nhs
                        if nhs == 0:
                            fo_cur[0] = foutp.tile(
                                [128, HID // 2], bf16, tag="fo", name="fo_i"
                            )
                        psF = ps3.tile([128, 512], f32, tag="psF")
                        for hl in range(HPC):
                            nc.tensor.matmul(
                                psF,
                                outT_sb[:, hl, tt * 128 : (tt + 1) * 128],
                                wo_sb[:, hl, nh * 512 : (nh + 1) * 512],
                                start=(hl == 0),
                                stop=(hl == HPC - 1),
                            )
                        nc.vector.tensor_copy(
                            fo_cur[0][:, nhs * 512 : (nhs + 1) * 512], psF
                        )
                        if nhs == 1:
                            nc.sync.dma_start(
                                out=out_p[
                                    t0 + tt * 128 : t0 + (tt + 1) * 128,
                                    hw * 1024 : (hw + 1) * 1024,
                                ],
                                in_=fo_cur[0],
                            )

                    for qci in range(S // QC):
                        q0 = qci * QC
                        for hl in range(HPC):
                            qTap = qk_sb[:, hl, :]
                            kTap = qk_sb[:, 2 + hl, :]
                            psO = ps2o.tile([128, QC], f32, tag="psO")
                            psD = ps2d.tile([128, QC], f32, tag="psD")
                            for kg in range(nkt // 2):
                                pe = pp.tile([128, 2, QC], fr, tag="pexp")
                                for j in range(2):
                                    kt = kg * 2 + j
                                    psS = ps2s.tile([128, QC], f32, tag="psS")
                                    nc.tensor.matmul(
                                        psS,
                                        kTap[:, kt * 128 : (kt + 1) * 128],
                                        qTap[:, q0 : q0 + QC],
                                        skip_group_check=True,
                                    )
                                    nc.scalar.activation(
                                        pe[:, j, :], psS, Exp, scale=SCALE
                                    )
                                for j in range(2):
                                    kt = kg * 2 + j
                                    first = kt == 0
                                    last = kt == nkt - 1
                                    nc.tensor.matmul(
                                        psO,
                                        v_sb[:, kt, hl * DH : (hl + 1) * DH],
                                        pe[:, j, :],
                                        start=first,
                                        stop=last,
                                        skip_group_check=True,
                                    )
                                    nc.tensor.matmul(
                                        psD,
                                        ones_sb,
                                        pe[:, j, :],
                                        start=first,
                                        stop=last,
                                        skip_group_check=True,
                                    )
                                if qci > 0:
                                    ph3_group(qci - 1, hl * 8 + kg)
                            rd = smallp.tile([128, QC], f32, tag="rd")
                            nc.vector.reciprocal_approx_fast(rd, psD)
                            nc.vector.tensor_mul(
                                outT_sb[:, hl, q0 : q0 + QC], psO, rd
                            )
                    for g in range(16):
                        ph3_group(S // QC - 1, g)

            for b in range(B):
                t0 = b * S

                # ---- phase 1: QKV projection (+ fused RoPE for q,k) ----
                # qk_sb rows: [q_h0, q_h1, k_h0, k_h1], each [128 d, S]
                qk_sb = qkvp.tile([128, 4, S], fr, tag="qk")
                v_sb = qkvp.tile([128, KT, OC], fr, tag="v")
                with tc.tile_pool(name="ps1", bufs=4, space="PSUM") as ps1:
                    # dispatch each chunk's load one iteration ahead so it
                    # never queues behind the RoPE swap DMAs on sync
                    hch_tiles = {}

                    def load_hch(tci):
                        soff = tci * TC
                        h = hpool.tile([128, KT, TC], fr, tag="hch", name="hch")
                        nc.sync.dma_start(
                            out=h, in_=hT_r[:, :, t0 + soff : t0 + soff + TC]
                        )
                        hch_tiles[tci] = h

                    load_hch(0)
                    for tci in range(S // TC):
                        soff = tci * TC
                        if tci + 1 < S // TC:
                            load_hch(tci + 1)
                        hch = hch_tiles.pop(tci)
                        for ot in range(4):
                            wsb, hl = w_of[ot]
                            ps = ps1.tile([128, TC], f32, tag="ps_qk")
                            for kk in range(KT):
                                nc.tensor.matmul(
                                    ps,
                                    wsb[:, kk, hl * DH : (hl + 1) * DH],
                                    hch[:, kk, :],
                                    start=(kk == 0),
                                    stop=(kk == KT - 1),
                                )
                            # RoPE: dst = ps*cc + blockswap(ps)*ss_signed
                            dst = qk_sb[:, ot, soff : soff + TC]
                            if R_ROPE:
                                # DVE reads PSUM directly, cross-partition
                                t2 = ropep.tile([128, TC], f32, tag="t2")
                                nc.vector.tensor_mul(
                                    t2[0:64, :],
                                    ps[64:128, :],
                                    ss_sb[0:64, soff : soff + TC],
                                )
                                nc.vector.tensor_mul(
                                    t2[64:128, :],
                                    ps[0:64, :],
                                    ss_sb[64:128, soff : soff + TC],
                                )
                                t1 = ropep.tile([128, TC], f32, tag="t1")
                                nc.vector.tensor_mul(
                                    t1, ps, cc_sb[:, soff : soff + TC]
                                )
                                nc.vector.tensor_add(dst, t1, t2)
                            else:
                                raw = ropep.tile([128, TC], f32, tag="raw")
                                nc.scalar.copy(raw, ps)
                                swp = ropep.tile([128, TC], f32, tag="swp")
                                nc.sync.dma_start(
                                    out=swp[0:64, :], in_=raw[64:128, :]
                                )
                                nc.sync.dma_start(
                                    out=swp[64:128, :], in_=raw[0:64, :]
                                )
                                t1 = ropep.tile([128, TC], f32, tag="t1")
                                nc.vector.tensor_mul(
                                    t1, raw, cc_sb[:, soff : soff + TC]
                                )
                                nc.vector.tensor_mul(
                                    swp, swp, ss_sb[:, soff : soff + TC]
                                )
                                nc.vector.tensor_add(dst, t1, swp)
                        for tt in range(TC // 128):
                            psv = ps1.tile([128, OC], f32, tag="ps_v")
                            for kk in range(KT):
                                nc.tensor.matmul(
                                    psv,
                                    hch[:, kk, tt * 128 : (tt + 1) * 128],
                                    wv_sb[:, kk, :],
                                    start=(kk == 0),
                                    stop=(kk == KT - 1),
                                )
                            nc.scalar.copy(
                                v_sb[:, tci * (TC // 128) + tt, :], psv
                            )

                # ---- phase 2: attention per head ----
                outT_sb = qkvp.tile([128, HPC, S], fr, tag="outT")
                if R_ILV:
                    phase23_ilv(t0, qk_sb, v_sb, outT_sb)
                    continue
                with (
                    tc.tile_pool(
                        name="ps2s", bufs=(2 if R_EXP2 else 4), space="PSUM"
                    ) as ps2s,
                    tc.tile_pool(name="ps2o", bufs=2, space="PSUM") as ps2o,
                    tc.tile_pool(name="ps2d", bufs=2, space="PSUM") as ps2d,
                ):
                    for hl in range(HPC):
                        qTap = qk_sb[:, hl, :]
                        kTap = qk_sb[:, 2 + hl, :]
                        for qci in range(S // QC):
                            q0 = qci * QC
                            psO = ps2o.tile([128, QC], f32, tag="psO")
                            psD = ps2d.tile([128, QC], f32, tag="psD")
                            nkt = S // 128
                            for kg in range(nkt // 2):
                                if R_EXP2:
                                    # two score tiles into one 2-bank PSUM
                                    # tile, one Exp drain over both banks
                                    psS = ps2s.tile([128, 2, QC], f32, tag="psS")
                                    for j in range(2):
                                        kt = kg * 2 + j
                                        nc.tensor.matmul(
                                            psS[:, j, :],
                                            kTap[:, kt * 128 : (kt + 1) * 128],
                                            qTap[:, q0 : q0 + QC],
                                            skip_group_check=True,
                                        )
                                    pe = pp.tile([128, 2, QC], fr, tag="pexp")
                                    nc.scalar.activation(pe, psS, Exp, scale=SCALE)
                                else:
                                    pe = pp.tile([128, 2, QC], fr, tag="pexp")
                                    for j in range(2):
                                        kt = kg * 2 + j
                                        psS = ps2s.tile(
                                            [128, QC], f32, tag="psS"
                                        )
                                        nc.tensor.matmul(
                                            psS,
                                            kTap[:, kt * 128 : (kt + 1) * 128],
                                            qTap[:, q0 : q0 + QC],
                                            skip_group_check=True,
                                        )
                                        nc.scalar.activation(
                                            pe[:, j, :], psS, Exp, scale=SCALE
                                        )
                                for j in range(2):
                                    kt = kg * 2 + j
                                    first = kt == 0
                                    last = kt == nkt - 1
                                    nc.tensor.matmul(
                                        psO,
                                        v_sb[:, kt, hl * DH : (hl + 1) * DH],
                                        pe[:, j, :],
                                        start=first,
                                        stop=last,
                                        skip_group_check=True,
                                    )
                                    nc.tensor.matmul(
                                        psD,
                                        ones_sb,
                                        pe[:, j, :],
                                        start=first,
                                        stop=last,
                                        skip_group_check=True,
                                    )
                            rd = smallp.tile([128, QC], f32, tag="rd")
                            if R_RECIP:
                                nc.vector.reciprocal_approx_fast(rd, psD)
                            else:
                                nc.vector.reciprocal(rd, psD)
                            nc.vector.tensor_mul(
                                outT_sb[:, hl, q0 : q0 + QC], psO, rd
                            )

                # ---- phase 3: output projection (partial over this core's heads) ----
                with tc.tile_pool(name="ps3", bufs=4, space="PSUM") as ps3:
                    for tt in range(S // 128):
                        for hw in range(2):
                            fo = foutp.tile([128, HID // 2], bf16, tag="fo")
                            for nhs in range(2):
                                nh = hw * 2 + nhs
                                psF = ps3.tile([128, 512], f32, tag="psF")
                                for hl in range(HPC):
                                    nc.tensor.matmul(
                                        psF,
                                        outT_sb[:, hl, tt * 128 : (tt + 1) * 128],
                                        wo_sb[:, hl, nh * 512 : (nh + 1) * 512],
                                        start=(hl == 0),
                                        stop=(hl == HPC - 1),
                                    )
                                nc.vector.tensor_copy(
                                    fo[:, nhs * 512 : (nhs + 1) * 512], psF
                                )
                            nc.sync.dma_start(
                                out=out_p[
                                    t0 + tt * 128 : t0 + (tt + 1) * 128,
                                    hw * 1024 : (hw + 1) * 1024,
                                ],
                                in_=fo,
                            )

    nc.compile()
    return nc


def _deint(idx128):
    """de-interleave a [128] index block: evens then odds."""
    return np.concatenate([idx128[0::2], idx128[1::2]])


def _prep_inputs(hidden_states, cos, sin, w_qkv, w_o):
    """Host-side shard/layout prep. Returns per-core input maps."""
    hs = np.ascontiguousarray(
        hidden_states.reshape(T, HID).T, dtype=np.float32
    )  # [HID, T]
    ccf = np.ascontiguousarray(
        np.concatenate([cos.T[0::2, :], cos.T[1::2, :]], axis=0), dtype=np.float32
    )  # [128, S] de-interleaved
    ssf = np.ascontiguousarray(
        np.concatenate([-sin.T[0::2, :], sin.T[1::2, :]], axis=0), dtype=np.float32
    )  # [128, S] de-interleaved, sign folded

    in_maps = []
    for c in range(NC):
        heads = [HPC * c + i for i in range(HPC)]
        qrows = np.concatenate([_deint(np.arange(h * DH, (h + 1) * DH)) for h in heads])
        krows = H * DH + qrows
        vrows = (
            np.concatenate([np.arange(h * DH, (h + 1) * DH) for h in heads])
            + 2 * H * DH
        )
        ocols = np.concatenate([np.arange(h * DH, (h + 1) * DH) for h in heads])
        in_maps.append(
            {
                "hT": hs,
                "wqT": np.ascontiguousarray(w_qkv[qrows, :].T, dtype=np.float32),
                "wkT": np.ascontiguousarray(w_qkv[krows, :].T, dtype=np.float32),
                "wvT": np.ascontiguousarray(w_qkv[vrows, :].T, dtype=np.float32),
                "woT": np.ascontiguousarray(w_o[:, ocols].T, dtype=np.float32),
                "cc": ccf,
                "ss": ssf,
            }
        )
    return in_maps


def kernel(hidden_states, cos, sin, w_qkv, w_o):
    global _exec_time_ns
    from concourse.bass_utils import run_bass_kernel_spmd

    hidden_states = np.asarray(hidden_states, dtype=np.float32)
    cos = np.asarray(cos, dtype=np.float32)
    sin = np.asarray(sin, dtype=np.float32)
    w_qkv = np.asarray(w_qkv, dtype=np.float32)
    w_o = np.asarray(w_o, dtype=np.float32)

    nc = _build()
    in_maps = _prep_inputs(hidden_states, cos, sin, w_qkv, w_o)
    res = run_bass_kernel_spmd(
        nc,
        in_maps,
        core_ids=list(range(NC)),
        trace=bool(int(os.environ.get("KERNEL_TRACE", "0"))),
    )
    _exec_time_ns = res.exec_time_ns
    globals()["_last_result"] = res

    acc = res.results[0]["out_p"].astype(np.float32)
    for c in range(1, NC):
        acc = acc + res.results[c]["out_p"].astype(np.float32)
    return acc.reshape(B, S, HID)
